# revision 7
# baseline (speedup 1.0000x reference)
"""ArcFace loss kernel for 8 Trainium2 NeuronCores (class-parallel).

Reference computation (B=2048, C=50000, D=128):
    e  = l2norm(x, axis=1);  Wn = l2norm(W, axis=1)
    wf = e @ Wn.T + b                       # [B, C]
    prediction = softmax(wf, axis=1)
    cos_theta_m = cos(acos(clip(wf)) + M)   # label column only
    logits = wf with label col replaced by cos_theta_m, * S
    loss = mean CE(logits, labels)

Sharding: classes split across 8 cores (6250 each, padded to 6272 =
49*128; padded rows get a unit weight vector and bias -1e4 so their
exp() is exactly 0).  Math is class-on-partition: each PSUM tile is
[128 classes, 512 batch] = WnT_tile.T @ eT_chunk; ScalarE turns it into
softmax numerators u = exp(wf + b) (bias is ACT's per-partition bias)
and v = exp(S*(wf + b)); TensorE ones-matmuls column-sum u and v into
per-batch denominators accumulated in PSUM across the 49 class tiles.
One AllReduce per batch quarter combines denominators + label-margin
terms; the cached bf16 numerators are then scaled by 1/d1 (broadcast
via a rank-1 ones outer-product) and written class-major; the host
transposes at unshard.  Label-column values are computed redundantly on
every core from host-gathered W[labels]; only core 0's copy enters the
AllReduce (central flag).  cos(acos(t)+M) = t*cos(M) - sqrt(1-t^2)*sin(M).

NOTE: dual-output instructions (activation accum_out,
tensor_tensor_reduce) crash this environment's HW path — avoided.
"""

import math
import os
import sys

import numpy as np

for _p in ("/opt/trn_rl_repo", "/root/.axon_site/_ro/trn_rl_repo"):
    if os.path.isdir(_p) and _p not in sys.path:
        sys.path.append(_p)

import concourse.bass as bass  # noqa: E402
import concourse.bacc as bacc  # noqa: E402
import concourse.mybir as mybir  # noqa: E402
import concourse.tile as tile  # noqa: E402
from concourse import bass_utils  # noqa: E402

FP32 = mybir.dt.float32
BF16 = mybir.dt.bfloat16
AXL = mybir.AxisListType
ALU = mybir.AluOpType
ACTF = mybir.ActivationFunctionType

B, D, C, NCORES = 2048, 128, 50000, 8
CS = C // NCORES            # 6250 real classes per core
NCT = 49                    # padded class tiles of 128
CSP = NCT * 128             # 6272
NBT = B // 128              # 16
ROUNDS = 4                  # AllReduce rounds (one per batch quarter)
BTR = NBT // ROUNDS         # 4 label-tile columns per round
BQ = B // ROUNDS            # 512 batch rows per round
S, MARGIN, EPS = 20.0, 0.1, 1e-7
PAD_B = -10000.0            # bias of padded classes -> exp == 0.0

_CACHED = {}


def _ts(i, n):
    return slice(i * n, (i + 1) * n)


def build():
    if "nc" in _CACHED:
        return _CACHED["nc"]

    nc = bacc.Bacc(
        "TRN2",
        target_bir_lowering=False,
        debug=False,
        enable_asserts=True,
        num_devices=NCORES,
    )

    x_d = nc.dram_tensor("x", [B, D], FP32, kind="ExternalInput").ap()
    w_d = nc.dram_tensor("w", [CSP, D], FP32, kind="ExternalInput").ap()
    b_d = nc.dram_tensor("b", [128, NCT], FP32, kind="ExternalInput").ap()
    wl_d = nc.dram_tensor("wlab", [B, D], FP32, kind="ExternalInput").ap()
    bl_d = nc.dram_tensor("blab", [128, NBT], FP32, kind="ExternalInput").ap()
    cen_d = nc.dram_tensor("central", [128, 1], FP32, kind="ExternalInput").ap()
    id_d = nc.dram_tensor("ident", [128, 128], FP32, kind="ExternalInput").ap()

    pred_d = nc.dram_tensor("pred", [CSP, B], FP32, kind="ExternalOutput").ap()
    loss_d = nc.dram_tensor("loss", [1, 1], FP32, kind="ExternalOutput").ap()

    with tile.TileContext(nc) as tc:
        _kernel_body(nc, tc, x_d, w_d, b_d, wl_d, bl_d, cen_d, id_d, pred_d, loss_d)

    nc.compile()
    _CACHED["nc"] = nc
    return nc


def _kernel_body(nc, tc, x_d, w_d, b_d, wl_d, bl_d, cen_d, id_d, pred_d, loss_d):
    cosm = float(math.cos(MARGIN))
    sinm = float(math.sin(MARGIN))

    with (
        tc.tile_pool(name="persist", bufs=1) as pp,
        tc.tile_pool(name="psum", bufs=3, space="PSUM") as psp,
        tc.tile_pool(name="psacc", bufs=2, space="PSUM") as psa,
        tc.tile_pool(name="psbc", bufs=1, space="PSUM") as psb,
        tc.tile_pool(name="dram", bufs=2, space="DRAM") as dp,
    ):
        # ---------------- persistent tiles ----------------
        eT = pp.tile([128, B], FP32)        # x normalized, transposed [D, B]
        WnT = pp.tile([128, CSP], FP32)     # W normalized, transposed [D, Cs]
        b_t = pp.tile([128, NCT], FP32)     # bias, per class tile column
        sb_t = pp.tile([128, NCT], FP32)    # S * bias
        ident_s = pp.tile([128, 128], FP32)
        cen_s = pp.tile([128, 1], FP32)
        blab_s = pp.tile([128, NBT], FP32)
        ones_bf = pp.tile([128, 1], BF16)   # lhsT of column-sum matmuls
        ones_r = pp.tile([1, 128], FP32)    # lhsT of 1/d1 broadcast
        wfl = pp.tile([128, NBT], FP32)     # label-column wf (pre-bias)
        delta = pp.tile([128, NBT], FP32)   # (exp(S cosm) - exp(S wf_lab)) * central
        marg = pp.tile([128, NBT], FP32)    # S * cos_theta_m * central
        qparts = pp.tile([1, ROUNDS], FP32)  # per-round sum(ln d2 - margin)

        nc.sync.dma_start(b_t, b_d)
        nc.sync.dma_start(ident_s, id_d)
        nc.sync.dma_start(cen_s, cen_d)
        nc.sync.dma_start(blab_s, bl_d)
        nc.vector.memset(ones_bf, 1.0)
        nc.vector.memset(ones_r, 1.0)
        nc.vector.tensor_scalar_mul(sb_t, b_t, S)

        # ---------------- prep: normalize + transpose x, W; label dots ---
        with tc.tile_pool(name="prep", bufs=1) as prep, tc.tile_pool(
            name="prepw", bufs=3
        ) as prepw:
            xall = prep.tile([128, B], FP32)
            nc.sync.dma_start(
                xall.rearrange("p (t d) -> p t d", d=D),
                x_d.rearrange("(t p) d -> p t d", p=128),
            )
            wlall = prep.tile([128, B], FP32)
            nc.sync.dma_start(
                wlall.rearrange("p (t d) -> p t d", d=D),
                wl_d.rearrange("(t p) d -> p t d", p=128),
            )
            wall = prep.tile([128, CSP], FP32)
            nc.sync.dma_start(
                wall.rearrange("p (t d) -> p t d", d=D),
                w_d.rearrange("(t p) d -> p t d", p=128),
            )

            # row sums-of-squares: x (16) | wlab (16) | W (49)
            nsq = prep.tile([128, NBT + NBT + NCT], FP32)
            for i, (src, n) in enumerate(((xall, NBT), (wlall, NBT), (wall, NCT))):
                base = (0, NBT, 2 * NBT)[i]
                for t in range(n):
                    scr = prepw.tile([128, D], FP32, tag="scr")
                    nc.vector.tensor_mul(scr, src[:, _ts(t, D)], src[:, _ts(t, D)])
                    nc.vector.tensor_reduce(
                        nsq[:, base + t : base + t + 1], scr, axis=AXL.X, op=ALU.add
                    )

            nrm = prep.tile([128, NBT + NBT + NCT], FP32)
            nc.scalar.activation(nrm, nsq, ACTF.Sqrt)
            rin = prep.tile([128, NBT + NBT + NCT], FP32)
            nc.vector.reciprocal(rin, nrm)

            # x: scale rows, transpose into eT; label dot against wlab rows
            for bt in range(NBT):
                et = prepw.tile([128, D], FP32, tag="et")
                nc.vector.tensor_scalar_mul(et, xall[:, _ts(bt, D)], rin[:, bt : bt + 1])
                tp = psp.tile([128, 128], FP32, tag="mm")
                nc.tensor.transpose(tp, et, ident_s)
                nc.scalar.copy(eT[:, _ts(bt, 128)], tp)

                wnl = prepw.tile([128, D], FP32, tag="wnl")
                nc.vector.tensor_scalar_mul(
                    wnl, wlall[:, _ts(bt, D)], rin[:, NBT + bt : NBT + bt + 1]
                )
                scr2 = prepw.tile([128, D], FP32, tag="scr2")
                nc.vector.tensor_mul(scr2, et, wnl)
                nc.vector.tensor_reduce(
                    wfl[:, bt : bt + 1], scr2, axis=AXL.X, op=ALU.add
                )

            # W: scale rows, transpose into WnT
            for ct in range(NCT):
                wn = prepw.tile([128, D], FP32, tag="wn")
                nc.vector.tensor_scalar_mul(
                    wn, wall[:, _ts(ct, D)], rin[:, 2 * NBT + ct : 2 * NBT + ct + 1]
                )
                tp = psp.tile([128, 128], FP32, tag="mm")
                nc.tensor.transpose(tp, wn, ident_s)
                nc.scalar.copy(WnT[:, _ts(ct, 128)], tp)

            # ---- label-column margin math (all [128, 16], every core) ----
            wfl2 = prep.tile([128, NBT], FP32)
            nc.vector.tensor_add(wfl2, wfl, blab_s)          # + b[label]
            tcl = prep.tile([128, NBT], FP32)
            nc.vector.tensor_scalar(
                tcl, wfl2, -1.0 + EPS, 1.0 - EPS, ALU.max, ALU.min
            )
            qq = prep.tile([128, NBT], FP32)
            nc.vector.tensor_mul(qq, tcl, tcl)               # t^2
            s1 = prep.tile([128, NBT], FP32)
            nc.vector.tensor_scalar(s1, qq, -1.0, 1.0, ALU.mult, ALU.add)
            rr = prep.tile([128, NBT], FP32)
            nc.scalar.activation(rr, s1, ACTF.Sqrt)
            tcs = prep.tile([128, NBT], FP32)
            nc.vector.tensor_scalar_mul(tcs, tcl, cosm)
            cm = prep.tile([128, NBT], FP32)                 # cos(acos(t)+M)
            nc.vector.scalar_tensor_tensor(cm, rr, -sinm, tcs, ALU.mult, ALU.add)
            em = prep.tile([128, NBT], FP32)
            nc.scalar.activation(em, cm, ACTF.Exp, scale=S)
            el = prep.tile([128, NBT], FP32)
            nc.scalar.activation(el, wfl2, ACTF.Exp, scale=S)
            dd = prep.tile([128, NBT], FP32)
            nc.vector.tensor_sub(dd, em, el)
            nc.vector.tensor_scalar_mul(delta, dd, cen_s[:, 0:1])
            nc.vector.tensor_scalar(marg, cm, cen_s[:, 0:1], S, ALU.mult, ALU.mult)

        # ---------------- main: 4 rounds over batch quarters ----------------
        with (
            tc.tile_pool(name="cache", bufs=100) as cp,
            tc.tile_pool(name="stage", bufs=3) as stp,
            tc.tile_pool(name="vsc", bufs=3) as vp,
            tc.tile_pool(name="small", bufs=1) as sp,
        ):
            for rd in range(ROUNDS):
                d1acc = psa.tile([1, BQ], FP32, tag="d1acc")
                d2acc = psa.tile([1, BQ], FP32, tag="d2acc")
                ucs = []
                for ct in range(NCT):
                    ps = psp.tile([128, BQ], FP32, tag="mm")
                    nc.tensor.matmul(
                        ps, WnT[:, _ts(ct, 128)], eT[:, _ts(rd, BQ)],
                        start=True, stop=True,
                    )
                    uc = cp.tile([128, BQ], BF16, tag="uc")
                    ucs.append(uc)
                    nc.scalar.activation(uc, ps, ACTF.Exp, bias=b_t[:, ct : ct + 1])
                    vs = vp.tile([128, BQ], BF16, tag="vs")
                    nc.scalar.activation(
                        vs, ps, ACTF.Exp, scale=S, bias=sb_t[:, ct : ct + 1]
                    )
                    nc.tensor.matmul(
                        d1acc, ones_bf, uc, start=(ct == 0), stop=(ct == NCT - 1)
                    )
                    nc.tensor.matmul(
                        d2acc, ones_bf, vs, start=(ct == 0), stop=(ct == NCT - 1)
                    )

                d1row = sp.tile([1, BQ], FP32, tag="d1row")
                nc.vector.tensor_copy(d1row, d1acc)
                d2row = sp.tile([1, BQ], FP32, tag="d2row")
                nc.vector.tensor_copy(d2row, d2acc)

                # payload [4, 512]: d1 | d2 | delta | margin.  delta/marg are
                # [128, 4] tiles; batch row (within round) = c*128 + p.
                ar_in = dp.tile([4, BQ], FP32, tag="ari")
                ar_out = dp.tile([4, BQ], FP32, tag="aro", addr_space="Shared")
                nc.sync.dma_start(ar_in[0:1, :], d1row)
                nc.sync.dma_start(ar_in[1:2, :], d2row)
                nc.sync.dma_start(
                    ar_in[2:3, :].rearrange("o (c p) -> p (o c)", p=128),
                    delta[:, _ts(rd, BTR)],
                )
                nc.sync.dma_start(
                    ar_in[3:4, :].rearrange("o (c p) -> p (o c)", p=128),
                    marg[:, _ts(rd, BTR)],
                )
                nc.gpsimd.collective_compute(
                    "AllReduce",
                    ALU.add,
                    replica_groups=[list(range(NCORES))],
                    ins=[ar_in.opt()],
                    outs=[ar_out.opt()],
                )

                d1g = sp.tile([1, BQ], FP32, tag="d1g")
                nc.sync.dma_start(d1g, ar_out[0:1, :])
                rec1 = sp.tile([1, BQ], FP32, tag="rec1")
                nc.vector.reciprocal(rec1, d1g)
                pb = psb.tile([128, BQ], FP32, tag="bc")
                nc.tensor.matmul(pb, ones_r, rec1, start=True, stop=True)
                rb = sp.tile([128, BQ], FP32, tag="rb", bufs=2)
                nc.scalar.copy(rb, pb)

                # loss pieces: ln(d2 + delta) - margin, summed over the round
                d2g = sp.tile([1, BQ], FP32, tag="d2g")
                nc.sync.dma_start(d2g, ar_out[1:2, :])
                dlg = sp.tile([1, BQ], FP32, tag="dlg")
                nc.sync.dma_start(dlg, ar_out[2:3, :])
                mgg = sp.tile([1, BQ], FP32, tag="mgg")
                nc.sync.dma_start(mgg, ar_out[3:4, :])
                d2f = sp.tile([1, BQ], FP32, tag="d2f")
                nc.vector.tensor_add(d2f, d2g, dlg)
                ldr = sp.tile([1, BQ], FP32, tag="ldr")
                nc.scalar.activation(ldr, d2f, ACTF.Ln)
                qrow = sp.tile([1, BQ], FP32, tag="qrow")
                nc.vector.tensor_sub(qrow, ldr, mgg)
                nc.vector.tensor_reduce(
                    qparts[:, rd : rd + 1], qrow, axis=AXL.X, op=ALU.add
                )

                # scale cached numerators and write the class-major slab
                for ct in range(NCT):
                    stg = stp.tile([128, BQ], FP32, tag="stg")
                    nc.vector.tensor_mul(stg, ucs[ct], rb)
                    nc.sync.dma_start(
                        pred_d[_ts(ct, 128), _ts(rd, BQ)], stg
                    )

            # ---------------- loss tail ----------------
            qs = pp.tile([1, 1], FP32)
            nc.vector.tensor_reduce(qs, qparts, axis=AXL.X, op=ALU.add)
            lt = pp.tile([1, 1], FP32)
            nc.scalar.mul(lt, qs, 1.0 / B)
            nc.sync.dma_start(loss_d, lt)


def make_in_maps(x, W, b, labels):
    x = np.ascontiguousarray(np.asarray(x, dtype=np.float32))
    W = np.asarray(W, dtype=np.float32)
    b = np.asarray(b, dtype=np.float32)
    labels = np.asarray(labels)

    wlab = np.ascontiguousarray(W[labels])                     # [B, D]
    blab = np.ascontiguousarray(b[labels].reshape(NBT, 128).T.astype(np.float32))
    ident = np.eye(128, dtype=np.float32)

    in_maps = []
    for k in range(NCORES):
        wsh = np.zeros((CSP, D), dtype=np.float32)
        wsh[:CS] = W[k * CS : (k + 1) * CS]
        wsh[CS:, 0] = 1.0  # unit rows keep l2norm finite; bias PAD_B
        #                    still forces exp() == 0 for padded columns
        bsh = np.full(CSP, PAD_B, dtype=np.float32)
        bsh[:CS] = b[k * CS : (k + 1) * CS]
        bsh = np.ascontiguousarray(bsh.reshape(NCT, 128).T)    # [128, NCT]
        cen = np.full((128, 1), 1.0 if k == 0 else 0.0, dtype=np.float32)
        in_maps.append(
            {
                "x": x,
                "w": wsh,
                "b": bsh,
                "wlab": wlab,
                "blab": blab,
                "central": cen,
                "ident": ident,
            }
        )
    return in_maps


def run(x, W, b, labels, trace=False, **kwargs):
    nc = build()
    in_maps = make_in_maps(x, W, b, labels)
    res = bass_utils.run_bass_kernel_spmd(
        nc, in_maps, core_ids=list(range(NCORES)), trace=trace, **kwargs
    )
    pred = np.concatenate(
        [res.results[k]["pred"][:CS].T for k in range(NCORES)], axis=1
    )
    loss = np.float32(res.results[0]["loss"][0, 0])
    return (pred, loss), res


def kernel(x, W, b, labels):
    out, _ = run(x, W, b, labels)
    return out


# revision 8
# speedup vs baseline: 1.1903x; 1.1903x over previous
"""ArcFace loss kernel for 8 Trainium2 NeuronCores (class-parallel).

Reference computation (B=2048, C=50000, D=128):
    e  = l2norm(x, axis=1);  Wn = l2norm(W, axis=1)
    wf = e @ Wn.T + b                       # [B, C]
    prediction = softmax(wf, axis=1)
    cos_theta_m = cos(acos(clip(wf)) + M)   # label column only
    logits = wf with label col replaced by cos_theta_m, * S
    loss = mean CE(logits, labels)

Sharding: classes split across 8 cores (6250 each, padded to 6272 =
49*128; padded rows get a unit weight vector and bias -1e4 so their
exp() is exactly 0).  Math is class-on-partition: each PSUM tile is
[128 classes, 512 batch] = WnT_tile.T @ eT_chunk; ScalarE turns it into
softmax numerators u = exp(wf + b) (bias is ACT's per-partition bias)
and v = exp(S*(wf + b)); TensorE ones-matmuls column-sum u and v into
per-batch denominators accumulated in PSUM across the 49 class tiles.
One AllReduce per batch quarter combines denominators + label-margin
terms; the cached bf16 numerators are then scaled by 1/d1 (broadcast
via a rank-1 ones outer-product) and written class-major; the host
transposes at unshard.  Label-column values are computed redundantly on
every core from host-gathered W[labels]; only core 0's copy enters the
AllReduce (central flag).  cos(acos(t)+M) = t*cos(M) - sqrt(1-t^2)*sin(M).

NOTE: dual-output instructions (activation accum_out,
tensor_tensor_reduce) crash this environment's HW path — avoided.
"""

import math
import os
import sys

import numpy as np

for _p in ("/opt/trn_rl_repo", "/root/.axon_site/_ro/trn_rl_repo"):
    if os.path.isdir(_p) and _p not in sys.path:
        sys.path.append(_p)

import concourse.bass as bass  # noqa: E402
import concourse.bacc as bacc  # noqa: E402
import concourse.mybir as mybir  # noqa: E402
import concourse.tile as tile  # noqa: E402
from concourse import bass_utils  # noqa: E402

FP32 = mybir.dt.float32
BF16 = mybir.dt.bfloat16
AXL = mybir.AxisListType
ALU = mybir.AluOpType
ACTF = mybir.ActivationFunctionType

B, D, C, NCORES = 2048, 128, 50000, 8
CS = C // NCORES            # 6250 real classes per core
NCT = 49                    # padded class tiles of 128
CSP = NCT * 128             # 6272
NBT = B // 128              # 16
ROUNDS = 4                  # AllReduce rounds (one per batch quarter)
BTR = NBT // ROUNDS         # 4 label-tile columns per round
BQ = B // ROUNDS            # 512 batch rows per round
S, MARGIN, EPS = 20.0, 0.1, 1e-7
PAD_B = -10000.0            # bias of padded classes -> exp == 0.0

_CACHED = {}


def _ts(i, n):
    return slice(i * n, (i + 1) * n)


def build():
    if "nc" in _CACHED:
        return _CACHED["nc"]

    nc = bacc.Bacc(
        "TRN2",
        target_bir_lowering=False,
        debug=False,
        enable_asserts=True,
        num_devices=NCORES,
    )

    x_d = nc.dram_tensor("x", [B, D], FP32, kind="ExternalInput").ap()
    w_d = nc.dram_tensor("w", [CSP, D], FP32, kind="ExternalInput").ap()
    b_d = nc.dram_tensor("b", [128, NCT], FP32, kind="ExternalInput").ap()
    wl_d = nc.dram_tensor("wlab", [B, D], FP32, kind="ExternalInput").ap()
    bl_d = nc.dram_tensor("blab", [128, NBT], FP32, kind="ExternalInput").ap()
    cen_d = nc.dram_tensor("central", [128, 1], FP32, kind="ExternalInput").ap()
    id_d = nc.dram_tensor("ident", [128, 128], FP32, kind="ExternalInput").ap()

    pred_d = nc.dram_tensor("pred", [CSP, B], BF16, kind="ExternalOutput").ap()
    loss_d = nc.dram_tensor("loss", [1, 1], FP32, kind="ExternalOutput").ap()

    with tile.TileContext(nc) as tc:
        _kernel_body(nc, tc, x_d, w_d, b_d, wl_d, bl_d, cen_d, id_d, pred_d, loss_d)

    nc.compile()
    _CACHED["nc"] = nc
    return nc


def _kernel_body(nc, tc, x_d, w_d, b_d, wl_d, bl_d, cen_d, id_d, pred_d, loss_d):
    cosm = float(math.cos(MARGIN))
    sinm = float(math.sin(MARGIN))

    with (
        tc.tile_pool(name="persist", bufs=1) as pp,
        tc.tile_pool(name="psum", bufs=3, space="PSUM") as psp,
        tc.tile_pool(name="psacc", bufs=2, space="PSUM") as psa,
        tc.tile_pool(name="psbc", bufs=1, space="PSUM") as psb,
        tc.tile_pool(name="dram", bufs=2, space="DRAM") as dp,
    ):
        # ---------------- persistent tiles ----------------
        eT = pp.tile([128, B], BF16)        # x normalized, transposed [D, B]
        WnT = pp.tile([128, CSP], BF16)     # W normalized, transposed [D, Cs]
        b_t = pp.tile([128, NCT], FP32)     # bias, per class tile column
        sb_t = pp.tile([128, NCT], FP32)    # S * bias
        ident_s = pp.tile([128, 128], FP32)
        cen_s = pp.tile([128, 1], FP32)
        blab_s = pp.tile([128, NBT], FP32)
        ones_bf = pp.tile([128, 1], BF16)   # lhsT of column-sum matmuls
        ones_r = pp.tile([1, 128], FP32)    # lhsT of 1/d1 broadcast
        wfl = pp.tile([128, NBT], FP32)     # label-column wf (pre-bias)
        delta = pp.tile([128, NBT], FP32)   # (exp(S cosm) - exp(S wf_lab)) * central
        marg = pp.tile([128, NBT], FP32)    # S * cos_theta_m * central
        qparts = pp.tile([1, ROUNDS], FP32)  # per-round sum(ln d2 - margin)

        nc.sync.dma_start(b_t, b_d)
        nc.sync.dma_start(ident_s, id_d)
        nc.sync.dma_start(cen_s, cen_d)
        nc.sync.dma_start(blab_s, bl_d)
        nc.vector.memset(ones_bf, 1.0)
        nc.vector.memset(ones_r, 1.0)
        nc.vector.tensor_scalar_mul(sb_t, b_t, S)

        # ---------------- prep: normalize + transpose x, W; label dots ---
        with tc.tile_pool(name="prep", bufs=1) as prep, tc.tile_pool(
            name="prepw", bufs=3
        ) as prepw:
            xall = prep.tile([128, B], FP32)
            nc.sync.dma_start(
                xall.rearrange("p (t d) -> p t d", d=D),
                x_d.rearrange("(t p) d -> p t d", p=128),
            )
            wlall = prep.tile([128, B], FP32)
            nc.sync.dma_start(
                wlall.rearrange("p (t d) -> p t d", d=D),
                wl_d.rearrange("(t p) d -> p t d", p=128),
            )
            wall = prep.tile([128, CSP], FP32)
            nc.sync.dma_start(
                wall.rearrange("p (t d) -> p t d", d=D),
                w_d.rearrange("(t p) d -> p t d", p=128),
            )

            # row sums-of-squares: x (16) | wlab (16) | W (49)
            nsq = prep.tile([128, NBT + NBT + NCT], FP32)
            sqx = prep.tile([128, B], FP32)
            nc.vector.tensor_mul(sqx, xall, xall)
            nc.vector.tensor_reduce(
                nsq[:, 0:NBT], sqx.rearrange("p (t d) -> p t d", d=D),
                axis=AXL.X, op=ALU.add)
            sql = prep.tile([128, B], FP32)
            nc.vector.tensor_mul(sql, wlall, wlall)
            nc.vector.tensor_reduce(
                nsq[:, NBT : 2 * NBT], sql.rearrange("p (t d) -> p t d", d=D),
                axis=AXL.X, op=ALU.add)
            sqw = prep.tile([128, CSP], FP32)
            nc.vector.tensor_mul(sqw, wall, wall)
            nc.vector.tensor_reduce(
                nsq[:, 2 * NBT :], sqw.rearrange("p (t d) -> p t d", d=D),
                axis=AXL.X, op=ALU.add)

            nrm = prep.tile([128, NBT + NBT + NCT], FP32)
            nc.scalar.activation(nrm, nsq, ACTF.Sqrt)
            rin = prep.tile([128, NBT + NBT + NCT], FP32)
            nc.vector.reciprocal(rin, nrm)

            # x: scale rows, transpose into eT; label dot against wlab rows
            for bt in range(NBT):
                et = prepw.tile([128, D], FP32, tag="et")
                nc.vector.tensor_scalar_mul(et, xall[:, _ts(bt, D)], rin[:, bt : bt + 1])
                tp = psp.tile([128, 128], FP32, tag="mm")
                nc.tensor.transpose(tp, et, ident_s)
                nc.scalar.copy(eT[:, _ts(bt, 128)], tp)

                wnl = prepw.tile([128, D], FP32, tag="wnl")
                nc.vector.tensor_scalar_mul(
                    wnl, wlall[:, _ts(bt, D)], rin[:, NBT + bt : NBT + bt + 1]
                )
                scr2 = prepw.tile([128, D], FP32, tag="scr2")
                nc.vector.tensor_mul(scr2, et, wnl)
                nc.vector.tensor_reduce(
                    wfl[:, bt : bt + 1], scr2, axis=AXL.X, op=ALU.add
                )

            # W: scale rows, transpose into WnT
            for ct in range(NCT):
                wn = prepw.tile([128, D], FP32, tag="wn")
                nc.vector.tensor_scalar_mul(
                    wn, wall[:, _ts(ct, D)], rin[:, 2 * NBT + ct : 2 * NBT + ct + 1]
                )
                tp = psp.tile([128, 128], FP32, tag="mm")
                nc.tensor.transpose(tp, wn, ident_s)
                nc.vector.tensor_copy(WnT[:, _ts(ct, 128)], tp)

            # ---- label-column margin math (all [128, 16], every core) ----
            wfl2 = prep.tile([128, NBT], FP32)
            nc.vector.tensor_add(wfl2, wfl, blab_s)          # + b[label]
            tcl = prep.tile([128, NBT], FP32)
            nc.vector.tensor_scalar(
                tcl, wfl2, -1.0 + EPS, 1.0 - EPS, ALU.max, ALU.min
            )
            qq = prep.tile([128, NBT], FP32)
            nc.vector.tensor_mul(qq, tcl, tcl)               # t^2
            s1 = prep.tile([128, NBT], FP32)
            nc.vector.tensor_scalar(s1, qq, -1.0, 1.0, ALU.mult, ALU.add)
            rr = prep.tile([128, NBT], FP32)
            nc.scalar.activation(rr, s1, ACTF.Sqrt)
            tcs = prep.tile([128, NBT], FP32)
            nc.vector.tensor_scalar_mul(tcs, tcl, cosm)
            cm = prep.tile([128, NBT], FP32)                 # cos(acos(t)+M)
            nc.vector.scalar_tensor_tensor(cm, rr, -sinm, tcs, ALU.mult, ALU.add)
            em = prep.tile([128, NBT], FP32)
            nc.scalar.activation(em, cm, ACTF.Exp, scale=S)
            el = prep.tile([128, NBT], FP32)
            el_inst = nc.scalar.activation(el, wfl2, ACTF.Exp, scale=S)
            dd = prep.tile([128, NBT], FP32)
            nc.vector.tensor_sub(dd, em, el)
            nc.vector.tensor_scalar_mul(delta, dd, cen_s[:, 0:1])
            nc.vector.tensor_scalar(marg, cm, cen_s[:, 0:1], S, ALU.mult, ALU.mult)

        # ---------------- main: 4 rounds over batch quarters ----------------
        first_exp_inst = None
        with (
            tc.tile_pool(name="cache", bufs=100) as cp,
            tc.tile_pool(name="stage", bufs=3) as stp,
            tc.tile_pool(name="vsc", bufs=3) as vp,
            tc.tile_pool(name="small", bufs=1) as sp,
        ):
            for rd in range(ROUNDS):
                d1acc = psa.tile([1, BQ], FP32, tag="d1acc")
                d2acc = psa.tile([1, BQ], FP32, tag="d2acc")
                ucs = []
                for ct in range(NCT):
                    ps = psp.tile([128, BQ], FP32, tag="mm")
                    nc.tensor.matmul(
                        ps, WnT[:, _ts(ct, 128)], eT[:, _ts(rd, BQ)],
                        start=True, stop=True,
                    )
                    uc = cp.tile([128, BQ], BF16, tag="uc")
                    ucs.append(uc)
                    _ei = nc.scalar.activation(
                        uc, ps, ACTF.Exp, bias=b_t[:, ct : ct + 1]
                    )
                    if first_exp_inst is None:
                        first_exp_inst = _ei
                    vs = vp.tile([128, BQ], BF16, tag="vs")
                    nc.scalar.activation(
                        vs, ps, ACTF.Exp, scale=S, bias=sb_t[:, ct : ct + 1]
                    )
                    nc.tensor.matmul(
                        d1acc, ones_bf, uc, start=(ct == 0), stop=(ct == NCT - 1)
                    )
                    nc.tensor.matmul(
                        d2acc, ones_bf, vs, start=(ct == 0), stop=(ct == NCT - 1)
                    )

                d1row = sp.tile([1, BQ], FP32, tag="d1row")
                nc.vector.tensor_copy(d1row, d1acc)
                d2row = sp.tile([1, BQ], FP32, tag="d2row")
                nc.vector.tensor_copy(d2row, d2acc)

                # payload [4, 512]: d1 | d2 | delta | margin.  delta/marg are
                # [128, 4] tiles; batch row (within round) = c*128 + p.
                ar_in = dp.tile([4, BQ], FP32, tag="ari")
                ar_out = dp.tile([4, BQ], FP32, tag="aro", addr_space="Shared")
                nc.sync.dma_start(ar_in[0:1, :], d1row)
                nc.sync.dma_start(ar_in[1:2, :], d2row)
                nc.sync.dma_start(
                    ar_in[2:3, :].rearrange("o (c p) -> p (o c)", p=128),
                    delta[:, _ts(rd, BTR)],
                )
                nc.sync.dma_start(
                    ar_in[3:4, :].rearrange("o (c p) -> p (o c)", p=128),
                    marg[:, _ts(rd, BTR)],
                )
                nc.gpsimd.collective_compute(
                    "AllReduce",
                    ALU.add,
                    replica_groups=[list(range(NCORES))],
                    ins=[ar_in.opt()],
                    outs=[ar_out.opt()],
                )

                d1g = sp.tile([1, BQ], FP32, tag="d1g")
                nc.sync.dma_start(d1g, ar_out[0:1, :])
                rec1 = sp.tile([1, BQ], FP32, tag="rec1")
                nc.vector.reciprocal(rec1, d1g)
                pb = psb.tile([128, BQ], FP32, tag="bc")
                nc.tensor.matmul(pb, ones_r, rec1, start=True, stop=True)
                rb = sp.tile([128, BQ], BF16, tag="rb", bufs=2)
                nc.scalar.copy(rb, pb)

                # loss pieces: ln(d2 + delta) - margin, summed over the round
                d2g = sp.tile([1, BQ], FP32, tag="d2g")
                nc.sync.dma_start(d2g, ar_out[1:2, :])
                dlg = sp.tile([1, BQ], FP32, tag="dlg")
                nc.sync.dma_start(dlg, ar_out[2:3, :])
                mgg = sp.tile([1, BQ], FP32, tag="mgg")
                nc.sync.dma_start(mgg, ar_out[3:4, :])
                d2f = sp.tile([1, BQ], FP32, tag="d2f")
                nc.vector.tensor_add(d2f, d2g, dlg)
                ldr = sp.tile([1, BQ], FP32, tag="ldr")
                nc.scalar.activation(ldr, d2f, ACTF.Ln)
                qrow = sp.tile([1, BQ], FP32, tag="qrow")
                nc.vector.tensor_sub(qrow, ldr, mgg)
                nc.vector.tensor_reduce(
                    qparts[:, rd : rd + 1], qrow, axis=AXL.X, op=ALU.add
                )

                # scale cached numerators and write the class-major slab
                for ct in range(NCT):
                    stg = stp.tile([128, BQ], BF16, tag="stg")
                    nc.vector.tensor_mul(stg, ucs[ct], rb)
                    nc.sync.dma_start(
                        pred_d[_ts(ct, 128), _ts(rd, BQ)], stg
                    )

            # ---------------- loss tail ----------------
            qs = pp.tile([1, 1], FP32)
            nc.vector.tensor_reduce(qs, qparts, axis=AXL.X, op=ALU.add)
            lt = pp.tile([1, 1], FP32)
            nc.scalar.mul(lt, qs, 1.0 / B)
            nc.sync.dma_start(loss_d, lt)

        # keep the ACT table sets in two phases: sqrt-set first, exp-set after
        if first_exp_inst is not None:
            try:
                tile.add_dep_helper(
                    first_exp_inst, el_inst, sync=False,
                    reason="ACT table ordering: label exps before main exps",
                )
            except Exception:
                pass


def make_in_maps(x, W, b, labels):
    x = np.ascontiguousarray(np.asarray(x, dtype=np.float32))
    W = np.asarray(W, dtype=np.float32)
    b = np.asarray(b, dtype=np.float32)
    labels = np.asarray(labels)

    wlab = np.ascontiguousarray(W[labels])                     # [B, D]
    blab = np.ascontiguousarray(b[labels].reshape(NBT, 128).T.astype(np.float32))
    ident = np.eye(128, dtype=np.float32)

    in_maps = []
    for k in range(NCORES):
        wsh = np.zeros((CSP, D), dtype=np.float32)
        wsh[:CS] = W[k * CS : (k + 1) * CS]
        wsh[CS:, 0] = 1.0  # unit rows keep l2norm finite; bias PAD_B
        #                    still forces exp() == 0 for padded columns
        bsh = np.full(CSP, PAD_B, dtype=np.float32)
        bsh[:CS] = b[k * CS : (k + 1) * CS]
        bsh = np.ascontiguousarray(bsh.reshape(NCT, 128).T)    # [128, NCT]
        cen = np.full((128, 1), 1.0 if k == 0 else 0.0, dtype=np.float32)
        in_maps.append(
            {
                "x": x,
                "w": wsh,
                "b": bsh,
                "wlab": wlab,
                "blab": blab,
                "central": cen,
                "ident": ident,
            }
        )
    return in_maps


def run(x, W, b, labels, trace=False, **kwargs):
    nc = build()
    in_maps = make_in_maps(x, W, b, labels)
    res = bass_utils.run_bass_kernel_spmd(
        nc, in_maps, core_ids=list(range(NCORES)), trace=trace, **kwargs
    )
    pred = np.concatenate(
        [res.results[k]["pred"][:CS].T.astype(np.float32) for k in range(NCORES)], axis=1
    )
    loss = np.float32(res.results[0]["loss"][0, 0])
    return (pred, loss), res


def kernel(x, W, b, labels):
    out, _ = run(x, W, b, labels)
    return out


# revision 10
# speedup vs baseline: 1.1999x; 1.0081x over previous
"""ArcFace loss kernel for 8 Trainium2 NeuronCores (class-parallel).

Reference computation (B=2048, C=50000, D=128):
    e  = l2norm(x, axis=1);  Wn = l2norm(W, axis=1)
    wf = e @ Wn.T + b                       # [B, C]
    prediction = softmax(wf, axis=1)
    cos_theta_m = cos(acos(clip(wf)) + M)   # label column only
    logits = wf with label col replaced by cos_theta_m, * S
    loss = mean CE(logits, labels)

Sharding: classes split across 8 cores (6250 each, padded to 6272 =
49*128; padded rows get a unit weight vector and bias -1e4 so their
exp() is exactly 0).  Math is class-on-partition: each PSUM tile is
[128 classes, 512 batch] = WnT_tile.T @ eT_chunk; ScalarE turns it into
softmax numerators u = exp(wf + b) (bias is ACT's per-partition bias)
and v = exp(S*(wf + b)); TensorE ones-matmuls column-sum u and v into
per-batch denominators accumulated in PSUM across the 49 class tiles.
One AllReduce per batch quarter combines denominators + label-margin
terms; the cached bf16 numerators are then scaled by 1/d1 (broadcast
via a rank-1 ones outer-product) and written class-major; the host
transposes at unshard.  Label-column values are computed redundantly on
every core from host-gathered W[labels]; only core 0's copy enters the
AllReduce (central flag).  cos(acos(t)+M) = t*cos(M) - sqrt(1-t^2)*sin(M).

NOTE: dual-output instructions (activation accum_out,
tensor_tensor_reduce) crash this environment's HW path — avoided.
"""

import math
import os
import sys

import numpy as np

for _p in ("/opt/trn_rl_repo", "/root/.axon_site/_ro/trn_rl_repo"):
    if os.path.isdir(_p) and _p not in sys.path:
        sys.path.append(_p)

import concourse.bass as bass  # noqa: E402
import concourse.bacc as bacc  # noqa: E402
import concourse.mybir as mybir  # noqa: E402
import concourse.tile as tile  # noqa: E402
from concourse import bass_utils  # noqa: E402

FP32 = mybir.dt.float32
BF16 = mybir.dt.bfloat16
AXL = mybir.AxisListType
ALU = mybir.AluOpType
ACTF = mybir.ActivationFunctionType

B, D, C, NCORES = 2048, 128, 50000, 8
CS = C // NCORES            # 6250 real classes per core
NCT = 49                    # padded class tiles of 128
CSP = NCT * 128             # 6272
NBT = B // 128              # 16
ROUNDS = 4                  # AllReduce rounds (one per batch quarter)
BTR = NBT // ROUNDS         # 4 label-tile columns per round
BQ = B // ROUNDS            # 512 batch rows per round
S, MARGIN, EPS = 20.0, 0.1, 1e-7
PAD_B = -10000.0            # bias of padded classes -> exp == 0.0

_CACHED = {}


def _ts(i, n):
    return slice(i * n, (i + 1) * n)


def build():
    if "nc" in _CACHED:
        return _CACHED["nc"]

    nc = bacc.Bacc(
        "TRN2",
        target_bir_lowering=False,
        debug=False,
        enable_asserts=True,
        num_devices=NCORES,
    )

    x_d = nc.dram_tensor("x", [B, D], FP32, kind="ExternalInput").ap()
    w_d = nc.dram_tensor("w", [CSP, D], FP32, kind="ExternalInput").ap()
    b_d = nc.dram_tensor("b", [128, NCT], FP32, kind="ExternalInput").ap()
    wl_d = nc.dram_tensor("wlab", [B, D], FP32, kind="ExternalInput").ap()
    bl_d = nc.dram_tensor("blab", [128, NBT], FP32, kind="ExternalInput").ap()
    cen_d = nc.dram_tensor("central", [128, 1], FP32, kind="ExternalInput").ap()
    id_d = nc.dram_tensor("ident", [128, 128], FP32, kind="ExternalInput").ap()

    pred_d = nc.dram_tensor("pred", [CSP, B], BF16, kind="ExternalOutput").ap()
    loss_d = nc.dram_tensor("loss", [1, 1], FP32, kind="ExternalOutput").ap()

    with tile.TileContext(nc) as tc:
        _kernel_body(nc, tc, x_d, w_d, b_d, wl_d, bl_d, cen_d, id_d, pred_d, loss_d)

    nc.compile()
    _CACHED["nc"] = nc
    return nc


def _kernel_body(nc, tc, x_d, w_d, b_d, wl_d, bl_d, cen_d, id_d, pred_d, loss_d):
    cosm = float(math.cos(MARGIN))
    sinm = float(math.sin(MARGIN))

    with (
        tc.tile_pool(name="persist", bufs=1) as pp,
        tc.tile_pool(name="psum", bufs=3, space="PSUM") as psp,
        tc.tile_pool(name="psacc", bufs=2, space="PSUM") as psa,
        tc.tile_pool(name="psbc", bufs=1, space="PSUM") as psb,
        tc.tile_pool(name="dram", bufs=2, space="DRAM") as dp,
    ):
        # ---------------- persistent tiles ----------------
        eT = pp.tile([128, B], BF16)        # x normalized, transposed [D, B]
        WnT = pp.tile([128, CSP], BF16)     # W normalized, transposed [D, Cs]
        b_t = pp.tile([128, NCT], FP32)     # bias, per class tile column
        e1_b = pp.tile([128, NCT], BF16)    # exp(b), colsum weights for d1
        e2_b = pp.tile([128, NCT], BF16)    # exp(S b), colsum weights for d2
        e1_f = pp.tile([128, NCT], FP32)    # f32 copy of e1_b (same values)
        ident_s = pp.tile([128, 128], FP32)
        cen_s = pp.tile([128, 1], FP32)
        blab_s = pp.tile([128, NBT], FP32)
        ones_bf = pp.tile([128, 1], BF16)   # lhsT of column-sum matmuls
        ones_r = pp.tile([1, 128], FP32)    # lhsT of 1/d1 broadcast
        wfl = pp.tile([128, NBT], FP32)     # label-column wf (pre-bias)
        delta = pp.tile([128, NBT], FP32)   # (exp(S cosm) - exp(S wf_lab)) * central
        marg = pp.tile([128, NBT], FP32)    # S * cos_theta_m * central
        qparts = pp.tile([1, ROUNDS], FP32)  # per-round sum(ln d2 - margin)

        nc.sync.dma_start(b_t, b_d)
        nc.sync.dma_start(ident_s, id_d)
        nc.sync.dma_start(cen_s, cen_d)
        nc.sync.dma_start(blab_s, bl_d)
        nc.vector.memset(ones_bf, 1.0)
        nc.vector.memset(ones_r, 1.0)


        # ---------------- prep: normalize + transpose x, W; label dots ---
        # W is processed in 4 groups so the main loop can start as soon as
        # the first group's WnT columns are ready.
        WG = [(0, 13), (13, 25), (25, 37), (37, 49)]
        with tc.tile_pool(name="prep", bufs=1) as prep, tc.tile_pool(
            name="prepw", bufs=3
        ) as prepw, tc.tile_pool(name="prepg", bufs=2) as prepg:
            xall = prep.tile([128, B], FP32)
            nc.sync.dma_start(
                xall.rearrange("p (t d) -> p t d", d=D),
                x_d.rearrange("(t p) d -> p t d", p=128),
            )
            wlall = prep.tile([128, B], FP32)
            nc.sync.dma_start(
                wlall.rearrange("p (t d) -> p t d", d=D),
                wl_d.rearrange("(t p) d -> p t d", p=128),
            )

            # x rows: normalize, transpose into eT
            sqx = prep.tile([128, B], FP32)
            nc.vector.tensor_mul(sqx, xall, xall)
            nsqx = prep.tile([128, NBT], FP32)
            nc.vector.tensor_reduce(
                nsqx, sqx.rearrange("p (t d) -> p t d", d=D), axis=AXL.X, op=ALU.add
            )
            nrmx = prep.tile([128, NBT], FP32)
            nc.scalar.activation(nrmx, nsqx, ACTF.Sqrt)
            rinx = prep.tile([128, NBT], FP32)
            nc.vector.reciprocal(rinx, nrmx)

            # wlab rows: normalize (no transpose needed)
            sql = prep.tile([128, B], FP32)
            nc.vector.tensor_mul(sql, wlall, wlall)
            nsql = prep.tile([128, NBT], FP32)
            nc.vector.tensor_reduce(
                nsql, sql.rearrange("p (t d) -> p t d", d=D), axis=AXL.X, op=ALU.add
            )
            nrml = prep.tile([128, NBT], FP32)
            nc.scalar.activation(nrml, nsql, ACTF.Sqrt)
            rinl = prep.tile([128, NBT], FP32)
            nc.vector.reciprocal(rinl, nrml)

            for bt in range(NBT):
                et = prepw.tile([128, D], FP32, tag="et")
                nc.vector.tensor_scalar_mul(
                    et, xall[:, _ts(bt, D)], rinx[:, bt : bt + 1]
                )
                tp = psp.tile([128, 128], FP32, tag="mm")
                nc.tensor.transpose(tp, et, ident_s)
                nc.scalar.copy(eT[:, _ts(bt, 128)], tp)

                wnl = prepw.tile([128, D], FP32, tag="wnl")
                nc.vector.tensor_scalar_mul(
                    wnl, wlall[:, _ts(bt, D)], rinl[:, bt : bt + 1]
                )
                scr2 = prepw.tile([128, D], FP32, tag="scr2")
                nc.vector.tensor_mul(scr2, et, wnl)
                nc.vector.tensor_reduce(
                    wfl[:, bt : bt + 1], scr2, axis=AXL.X, op=ALU.add
                )

            # W groups: DMA, normalize, transpose into WnT
            wr = w_d.rearrange("(t p) d -> p t d", p=128)
            for g0, g1 in WG:
                gw = (g1 - g0) * D
                wg = prepg.tile([128, 13 * D], FP32, tag="wg")
                nc.sync.dma_start(
                    wg[:, :gw].rearrange("p (t d) -> p t d", d=D), wr[:, g0:g1, :]
                )
                sqw = prepg.tile([128, 13 * D], FP32, tag="sqw")
                nc.vector.tensor_mul(sqw[:, :gw], wg[:, :gw], wg[:, :gw])
                nsqw = prepg.tile([128, 13], FP32, tag="nsqw")
                nc.vector.tensor_reduce(
                    nsqw[:, : g1 - g0],
                    sqw[:, :gw].rearrange("p (t d) -> p t d", d=D),
                    axis=AXL.X, op=ALU.add,
                )
                nrmw = prepg.tile([128, 13], FP32, tag="nrmw")
                nc.scalar.activation(nrmw[:, : g1 - g0], nsqw[:, : g1 - g0], ACTF.Sqrt)
                rinw = prepg.tile([128, 13], FP32, tag="rinw")
                nc.vector.reciprocal(rinw[:, : g1 - g0], nrmw[:, : g1 - g0])
                for i, ct in enumerate(range(g0, g1)):
                    wn = prepw.tile([128, D], FP32, tag="wn")
                    nc.vector.tensor_scalar_mul(
                        wn, wg[:, _ts(i, D)], rinw[:, i : i + 1]
                    )
                    tp = psp.tile([128, 128], FP32, tag="mm")
                    nc.tensor.transpose(tp, wn, ident_s)
                    nc.scalar.copy(WnT[:, _ts(ct, 128)], tp)

            # ---- label-column margin math (all [128, 16], every core) ----
            wfl2 = prep.tile([128, NBT], FP32)
            nc.vector.tensor_add(wfl2, wfl, blab_s)          # + b[label]
            tcl = prep.tile([128, NBT], FP32)
            nc.vector.tensor_scalar(
                tcl, wfl2, -1.0 + EPS, 1.0 - EPS, ALU.max, ALU.min
            )
            qq = prep.tile([128, NBT], FP32)
            nc.vector.tensor_mul(qq, tcl, tcl)               # t^2
            s1 = prep.tile([128, NBT], FP32)
            nc.vector.tensor_scalar(s1, qq, -1.0, 1.0, ALU.mult, ALU.add)
            rr = prep.tile([128, NBT], FP32)
            nc.scalar.activation(rr, s1, ACTF.Sqrt)
            tcs = prep.tile([128, NBT], FP32)
            nc.vector.tensor_scalar_mul(tcs, tcl, cosm)
            cm = prep.tile([128, NBT], FP32)                 # cos(acos(t)+M)
            nc.vector.scalar_tensor_tensor(cm, rr, -sinm, tcs, ALU.mult, ALU.add)
            em = prep.tile([128, NBT], FP32)
            nc.scalar.activation(em, cm, ACTF.Exp, scale=S)
            el = prep.tile([128, NBT], FP32)
            el_inst = nc.scalar.activation(el, wfl2, ACTF.Exp, scale=S)
            dd = prep.tile([128, NBT], FP32)
            nc.vector.tensor_sub(dd, em, el)
            nc.vector.tensor_scalar_mul(delta, dd, cen_s[:, 0:1])
            nc.vector.tensor_scalar(marg, cm, cen_s[:, 0:1], S, ALU.mult, ALU.mult)

        # colsum weights (exp set; after all prep sqrts)
        nc.scalar.activation(e1_b, b_t, ACTF.Exp)
        nc.scalar.activation(e2_b, b_t, ACTF.Exp, scale=S)
        nc.scalar.copy(e1_f, e1_b)

        # ---------------- main: 4 rounds over batch quarters ----------------
        first_exp_inst = None
        with (
            tc.tile_pool(name="cache", bufs=100) as cp,
            tc.tile_pool(name="stage", bufs=3) as stp,
            tc.tile_pool(name="vsc", bufs=6) as vp,
            tc.tile_pool(name="small", bufs=1) as sp,
        ):
            for rd in range(ROUNDS):
                d1acc = psa.tile([1, BQ], FP32, tag="d1acc")
                d2acc = psa.tile([1, BQ], FP32, tag="d2acc")
                ucs = []
                vss = []

                def _colsums(cs):
                    nc.tensor.matmul(
                        d1acc, e1_b[:, cs : cs + 1], ucs[cs],
                        start=(cs == 0), stop=(cs == NCT - 1),
                    )
                    nc.tensor.matmul(
                        d2acc, e2_b[:, cs : cs + 1], vss[cs],
                        start=(cs == 0), stop=(cs == NCT - 1),
                    )

                LAG = 2
                for ct in range(NCT):
                    ps = psp.tile([128, BQ], FP32, tag="mm")
                    nc.tensor.matmul(
                        ps, WnT[:, _ts(ct, 128)], eT[:, _ts(rd, BQ)],
                        start=True, stop=True,
                    )
                    uc = cp.tile([128, BQ], BF16, tag="uc")
                    ucs.append(uc)
                    _ei = nc.scalar.activation(uc, ps, ACTF.Exp)
                    if first_exp_inst is None:
                        first_exp_inst = _ei
                    vs = vp.tile([128, BQ], BF16, tag="vs")
                    vss.append(vs)
                    nc.scalar.activation(vs, ps, ACTF.Exp, scale=S)
                    if ct >= LAG:
                        _colsums(ct - LAG)
                for cs in range(NCT - LAG, NCT):
                    _colsums(cs)

                d1row = sp.tile([1, BQ], FP32, tag="d1row")
                nc.vector.tensor_copy(d1row, d1acc)
                d2row = sp.tile([1, BQ], FP32, tag="d2row")
                nc.vector.tensor_copy(d2row, d2acc)

                # payload [4, 512]: d1 | d2 | delta | margin.  delta/marg are
                # [128, 4] tiles; batch row (within round) = c*128 + p.
                ar_in = dp.tile([4, BQ], FP32, tag="ari")
                ar_out = dp.tile([4, BQ], FP32, tag="aro", addr_space="Shared")
                nc.sync.dma_start(ar_in[0:1, :], d1row)
                nc.sync.dma_start(ar_in[1:2, :], d2row)
                nc.sync.dma_start(
                    ar_in[2:3, :].rearrange("o (c p) -> p (o c)", p=128),
                    delta[:, _ts(rd, BTR)],
                )
                nc.sync.dma_start(
                    ar_in[3:4, :].rearrange("o (c p) -> p (o c)", p=128),
                    marg[:, _ts(rd, BTR)],
                )
                nc.gpsimd.collective_compute(
                    "AllReduce",
                    ALU.add,
                    replica_groups=[list(range(NCORES))],
                    ins=[ar_in.opt()],
                    outs=[ar_out.opt()],
                )

                d1g = sp.tile([1, BQ], FP32, tag="d1g")
                nc.sync.dma_start(d1g, ar_out[0:1, :])
                rec1 = sp.tile([1, BQ], FP32, tag="rec1")
                nc.vector.reciprocal(rec1, d1g)
                pb = psb.tile([128, BQ], FP32, tag="bc")
                nc.tensor.matmul(pb, ones_r, rec1, start=True, stop=True)
                rb = sp.tile([128, BQ], FP32, tag="rb", bufs=2)
                nc.scalar.copy(rb, pb)

                # loss pieces: ln(d2 + delta) - margin, summed over the round
                d2g = sp.tile([1, BQ], FP32, tag="d2g")
                nc.sync.dma_start(d2g, ar_out[1:2, :])
                dlg = sp.tile([1, BQ], FP32, tag="dlg")
                nc.sync.dma_start(dlg, ar_out[2:3, :])
                mgg = sp.tile([1, BQ], FP32, tag="mgg")
                nc.sync.dma_start(mgg, ar_out[3:4, :])
                d2f = sp.tile([1, BQ], FP32, tag="d2f")
                nc.vector.tensor_add(d2f, d2g, dlg)
                ldr = sp.tile([1, BQ], FP32, tag="ldr")
                nc.scalar.activation(ldr, d2f, ACTF.Ln)
                qrow = sp.tile([1, BQ], FP32, tag="qrow")
                nc.vector.tensor_sub(qrow, ldr, mgg)
                nc.vector.tensor_reduce(
                    qparts[:, rd : rd + 1], qrow, axis=AXL.X, op=ALU.add
                )

                # scale cached numerators and write the class-major slab
                for ct in range(NCT):
                    stg = stp.tile([128, BQ], BF16, tag="stg")
                    nc.vector.scalar_tensor_tensor(
                        stg, ucs[ct], e1_f[:, ct : ct + 1], rb, ALU.mult, ALU.mult
                    )
                    nc.sync.dma_start(
                        pred_d[_ts(ct, 128), _ts(rd, BQ)], stg
                    )

            # ---------------- loss tail ----------------
            qs = pp.tile([1, 1], FP32)
            nc.vector.tensor_reduce(qs, qparts, axis=AXL.X, op=ALU.add)
            lt = pp.tile([1, 1], FP32)
            nc.scalar.mul(lt, qs, 1.0 / B)
            nc.sync.dma_start(loss_d, lt)

        # keep the ACT table sets in two phases: sqrt-set first, exp-set after
        if first_exp_inst is not None:
            try:
                tile.add_dep_helper(
                    first_exp_inst, el_inst, sync=False,
                    reason="ACT table ordering: label exps before main exps",
                )
            except Exception:
                pass


def make_in_maps(x, W, b, labels):
    x = np.ascontiguousarray(np.asarray(x, dtype=np.float32))
    W = np.asarray(W, dtype=np.float32)
    b = np.asarray(b, dtype=np.float32)
    labels = np.asarray(labels)

    wlab = np.ascontiguousarray(W[labels])                     # [B, D]
    blab = np.ascontiguousarray(b[labels].reshape(NBT, 128).T.astype(np.float32))
    ident = np.eye(128, dtype=np.float32)

    in_maps = []
    for k in range(NCORES):
        wsh = np.zeros((CSP, D), dtype=np.float32)
        wsh[:CS] = W[k * CS : (k + 1) * CS]
        wsh[CS:, 0] = 1.0  # unit rows keep l2norm finite; bias PAD_B
        #                    still forces exp() == 0 for padded columns
        bsh = np.full(CSP, PAD_B, dtype=np.float32)
        bsh[:CS] = b[k * CS : (k + 1) * CS]
        bsh = np.ascontiguousarray(bsh.reshape(NCT, 128).T)    # [128, NCT]
        cen = np.full((128, 1), 1.0 if k == 0 else 0.0, dtype=np.float32)
        in_maps.append(
            {
                "x": x,
                "w": wsh,
                "b": bsh,
                "wlab": wlab,
                "blab": blab,
                "central": cen,
                "ident": ident,
            }
        )
    return in_maps


def run(x, W, b, labels, trace=False, **kwargs):
    nc = build()
    in_maps = make_in_maps(x, W, b, labels)
    res = bass_utils.run_bass_kernel_spmd(
        nc, in_maps, core_ids=list(range(NCORES)), trace=trace, **kwargs
    )
    pred = np.concatenate(
        [res.results[k]["pred"][:CS].T.astype(np.float32) for k in range(NCORES)], axis=1
    )
    loss = np.float32(res.results[0]["loss"][0, 0])
    return (pred, loss), res


def kernel(x, W, b, labels):
    out, _ = run(x, W, b, labels)
    return out
